# revision 13
# baseline (speedup 1.0000x reference)
"""Trainium2 Bass kernel for nn_DecodeNFlowFunc (dense MLP normalizing-flow decode).

Strategy: pure data-parallel over 8 NeuronCores (batch 524288 -> 65536/core).
On-chip layout is feature-major ([feature partitions, sample columns]); the
tiny MLP weights are pre-transformed on the host into block-diagonal /
permutation-folded stationary matrices so each matmul streams 512 sample
columns at 1 cycle/column (float32r). The per-sample feature permutations are
PE matmuls against permutation matrices; the s-vector sum-augmentation
(concat(s, -sum(s))) is folded into a [64,63] "S-fold" matmul so no partition
reduction is needed.

Host/dispatch architecture: the axon PJRT tunnel serializes transfers at
~35 MB/s per client connection, so a single-process dispatch is wire-bound.
kernel() therefore keeps a persistent pool of 8 worker processes, one per
NeuronCore, each with its own jax/PJRT client (own tunnel connection). Each
worker compiles the single-core NEFF once (a file-locked on-disk cache shares
the expensive BIR->NEFF compile across workers), keeps weights and the output
zero-buffer resident on its device, and per call only uploads its 512 KB z
shard, runs, and fetches its 16 MB bf16 output shard. Output travels as bf16
(worst-case 0.4% per-element error vs the 2e-2 gate) and is widened to f32
into a shared-memory buffer by each worker.
"""

import atexit
import hashlib
import os
import pickle
import struct
import subprocess
import sys
import tempfile
import numpy as np
from multiprocessing import shared_memory

N_CORES = 8
N_TOTAL = 524288
NPC = N_TOTAL // N_CORES  # 65536 samples per core
SUPER = 2048              # samples per supertile (4 groups of 512)
TILE = 512

DIM_X, DIM_Z, N_BLK, DD, H = 128, 2, 4, 64, 32
SM1 = 63

Z_BYTES = N_TOTAL * DIM_Z * 4
OUT_BYTES = N_TOTAL * DIM_X * 4
N_ROT = 8  # rotating output buffers so returned views survive later calls
NEFF_CACHE_DIR = "/tmp/nflow_neff_cache"
JAX_CACHE_DIR = "/tmp/nflow_jax_cache"


# ---------------------------------------------------------------- walrus fix
def _fix_sync_limits(nc):
    """This container's walrus accepts at most ONE sync wait and ONE sync
    update per engine instruction. Split extras onto adjacent same-engine
    nops (engine streams are FIFO, so semantics are preserved)."""
    import bass_rust
    import concourse.mybir as mybir

    counter = [0]

    def mknop(engine, waits, updates):
        counter[0] += 1
        nop = mybir.InstNoOp(name=f"I-waitfix-{counter[0]}", ins=[], outs=[])
        nop.engine = engine
        nop.sync_info = bass_rust.SyncInfo(on_wait=waits, on_update=updates)
        return nop

    for fn in nc.m.functions:
        for blk in fn.blocks:
            insts = blk.instructions  # live list
            out = []
            for inst in list(insts):
                si = inst.sync_info
                pre, post = [], []
                if si is not None:
                    waits = list(si.on_wait)
                    if len(waits) > 1:
                        for w in waits[:-1]:
                            pre.append(mknop(inst.engine, [w], []))
                        si.on_wait = [waits[-1]]
                    updates = list(si.on_update)
                    if len(updates) > 1 and not isinstance(inst, mybir.InstDMACopy):
                        for u in updates[1:]:
                            post.append(mknop(inst.engine, [], [u]))
                        si.on_update = [updates[0]]
                out.extend(pre)
                out.append(inst)
                out.extend(post)
            if len(out) != len(insts):
                insts.clear()
                insts.extend(out)


# ------------------------------------------------------------- host weights
def _perms():
    ps = []
    for ii in range(N_BLK):
        np.random.seed(ii)
        ps.append(np.random.permutation(DIM_X))
    return np.stack(ps)


def _bd(m, g):
    """block-diag of m repeated g times: [g*r, g*c]"""
    r, c = m.shape
    out = np.zeros((g * r, g * c), np.float32)
    for i in range(g):
        out[i * r:(i + 1) * r, i * c:(i + 1) * c] = m
    return out


def _prep_weights(fw0, fb0, fw1, fb1, fw2, fb2, cw0, cb0, cw1, cb1, cw2, cb2):
    w = {}
    perms = _perms()
    w["wL1"] = fw0.T.astype(np.float32).copy()             # [2, 32]
    w["wL2"] = _bd(fw1.T.astype(np.float32), 4)            # [128, 128]
    wl3aug = np.zeros((34, 128), np.float32)
    wl3aug[0:32, 2:128] = fw2.T
    wl3aug[32, 0] = 1.0
    wl3aug[33, 1] = 1.0
    w["wL3"] = wl3aug                                      # [34, 128]
    w["bL1"] = np.tile(fb0, 4).astype(np.float32)[:, None]  # [128,1]
    w["bL2"] = np.tile(fb1, 4).astype(np.float32)[:, None]
    bl3aug = np.zeros(128, np.float32)
    bl3aug[2:128] = fb2
    w["bL3"] = bl3aug[:, None]                             # [128,1]
    for ii in range(N_BLK):
        P = np.zeros((DIM_X, DIM_X), np.float32)
        P[np.arange(DIM_X), perms[ii]] = 1.0               # y = P @ x
        w[f"wP{ii}"] = P.T.copy()                          # lhsT
    for k in range(2 * N_BLK):
        w[f"wC0_{k}"] = np.tile(cw0[k].T.astype(np.float32), (2, 1))  # [128,32]
        w[f"bC0_{k}"] = np.tile(cb0[k], 4).astype(np.float32)[:, None]
        w[f"wC1_{k}"] = _bd(cw1[k].T.astype(np.float32), 4)    # [128, 128]
        w[f"bC1_{k}"] = np.tile(cb1[k], 4).astype(np.float32)[:, None]
        w[f"wC2s_{k}"] = np.tile(_bd(cw2[k][:SM1].T.astype(np.float32), 2), (2, 1))  # [128,126]
        w[f"bC2s_{k}"] = np.tile(cb2[k][:SM1], 2).astype(np.float32)[:, None]
        w[f"wC2t_{k}"] = np.tile(_bd(cw2[k][SM1:].T.astype(np.float32), 2), (2, 1))  # [128,128]
        w[f"bC2t_{k}"] = np.tile(cb2[k][SM1:], 2).astype(np.float32)[:, None]
    # S-fold: s64 = 0.1 * [[I63],[-1]] @ tanh(st_s); lhsT = S.T -> [63, 64]
    S = np.concatenate([np.eye(SM1, dtype=np.float32),
                        -np.ones((1, SM1), np.float32)], axis=0) * 0.1  # [64,63]
    w["wSF"] = _bd(S.T, 2)                                 # [126, 128]
    w["ident"] = np.eye(DIM_X, dtype=np.float32)
    return w


def _wshapes():
    ws = {
        "wL1": [2, 32], "wL2": [128, 128], "wL3": [34, 128],
        "bL1": [128, 1], "bL2": [128, 1], "bL3": [128, 1],
        "wSF": [126, 128], "ident": [128, 128],
    }
    for ii in range(N_BLK):
        ws[f"wP{ii}"] = [128, 128]
    for k in range(2 * N_BLK):
        ws[f"wC0_{k}"] = [128, 32]
        ws[f"bC0_{k}"] = [128, 1]
        ws[f"wC1_{k}"] = [128, 128]
        ws[f"bC1_{k}"] = [128, 1]
        ws[f"wC2s_{k}"] = [128, 126]
        ws[f"bC2s_{k}"] = [126, 1]
        ws[f"wC2t_{k}"] = [128, 128]
        ws[f"bC2t_{k}"] = [128, 1]
    return ws


# --------------------------------------------------------------- bass build
def _build(npc):
    import concourse.bass as bass
    import concourse.mybir as mybir
    from concourse.tile import TileContext

    F32 = mybir.dt.float32
    F32R = mybir.dt.float32r
    BF16 = mybir.dt.bfloat16
    AF = mybir.ActivationFunctionType

    nc = bass.Bass()
    n_st = npc // SUPER

    z = nc.declare_dram_parameter("z", [npc, DIM_Z], F32R, isOutput=False)
    out = nc.declare_dram_parameter("out", [npc, DIM_X], BF16, isOutput=True)

    wshapes = _wshapes()
    wdram = {n: nc.declare_dram_parameter(n, s, F32 if n.startswith("b") else F32R,
                                          isOutput=False)
             for n, s in wshapes.items()}

    # z samples per supertile st: sample = 2048*st + 16*p + 4*q + u
    z_r = z.rearrange("(a p b) c -> a p (b c)", p=128, b=16)      # [n_st,128,32]
    out_r = out.rearrange("(a p g t) f -> a p g t f", p=128, g=4, t=4)

    from contextlib import ExitStack
    with TileContext(nc) as tc, ExitStack() as ctx:
        cpool = ctx.enter_context(tc.tile_pool(name="consts", bufs=1))
        wsb = {}
        for n, s in wshapes.items():
            t = cpool.tile(s, F32 if n.startswith("b") else F32R, tag=n)
            nc.sync.dma_start(out=t[:], in_=wdram[n][:])
            wsb[n] = t
        idr = wsb["ident"][:]

        work = ctx.enter_context(tc.tile_pool(name="work", bufs=3))
        xpool = ctx.enter_context(tc.tile_pool(name="xt", bufs=10))
        psA = ctx.enter_context(tc.tile_pool(name="psA", bufs=2, space="PSUM"))
        psB = ctx.enter_context(tc.tile_pool(name="psB", bufs=2, space="PSUM"))
        psC = ctx.enter_context(tc.tile_pool(name="psC", bufs=2, space="PSUM"))
        psT = ctx.enter_context(tc.tile_pool(name="psT", bufs=2, space="PSUM"))

        def mm(pt, w, rhs, **kw):
            if not isinstance(w, bass.AP):
                w = w[:]
            nc.tensor.matmul(pt, w, rhs, **kw)

        for st in range(n_st):
            # ---- load z; 16 [128,2] transposes -> four zTg [2, 512]
            z_nat = work.tile([128, 32], F32R, tag="z_nat")
            nc.sync.dma_start(out=z_nat[:], in_=z_r[st])
            zTs = []
            for g in range(4):
                zTgp = psC.tile([2, 512], F32, tag="pC")
                for w_ in range(4):
                    j = 4 * g + w_
                    nc.tensor.transpose(
                        zTgp[:, 128 * w_:128 * (w_ + 1)].bitcast(F32R),
                        z_nat[:, 2 * j:2 * j + 2], idr)
                zTg = work.tile([2, 512], F32R, tag="zTg")
                nc.scalar.activation(zTg[:], zTgp[:], AF.Copy)
                zTs.append(zTg)

            # ---- first MLP: L1 per group (K=2), packed into two PSUM tiles
            H1 = work.tile([128, 512], F32R, tag="H1")
            for g in range(4):
                h1pg = psB.tile([32, 512], F32, tag="c0")
                mm(h1pg[:], wsb["wL1"], zTs[g][:])
                nc.scalar.activation(H1[32 * g:32 * (g + 1), :], h1pg[:], AF.Relu,
                                     bias=wsb["bL1"][32 * g:32 * (g + 1), :])
            h2p = psA.tile([128, 512], F32, tag="pA")
            mm(h2p[:], wsb["wL2"], H1[:])

            # ---- per group: H2aug = [relu(h2); zT] then augmented L3 -> X
            X = []
            for u in range(4):
                H2aug = work.tile([34, 512], F32R, tag="H2aug")
                nc.scalar.activation(H2aug[0:32, :], h2p[32 * u:32 * (u + 1), :],
                                     AF.Relu, bias=wsb["bL2"][32 * u:32 * (u + 1), :])
                nc.vector.tensor_copy(H2aug[32:34, :], zTs[u][:])
                xp = psA.tile([128, 512], F32, tag="pA")
                mm(xp[:], wsb["wL3"], H2aug[:])
                Xu = xpool.tile([128, 512], F32R, tag="X")
                nc.scalar.activation(Xu[:], xp[:], AF.Identity, bias=wsb["bL3"][:])
                X.append(Xu)

            # ---- 4 blocks x 2 couplings
            for ii in range(N_BLK):
                Y = []
                for u in range(4):
                    Yp = psA.tile([128, 512], F32, tag="pA")
                    mm(Yp[:], wsb[f"wP{ii}"], X[u][:])
                    Yu = xpool.tile([128, 512], F32R, tag="Y")
                    nc.scalar.activation(Yu[:], Yp[:], AF.Copy)
                    Y.append(Yu)
                Xn = []
                for _u in range(4):
                    Xnu = xpool.tile([128, 512], F32R, tag="X")
                    Xn.append(Xnu)
                for jj in range(2):
                    k = 2 * ii + jj
                    if jj == 0:
                        x1 = [Y[u][0:64, :] for u in range(4)]
                        x2 = [Y[u][64:128, :] for u in range(4)]
                        tdst = [Xn[u][64:128, :] for u in range(4)]
                    else:
                        x1 = [Xn[u][64:128, :] for u in range(4)]
                        x2 = [Y[u][0:64, :] for u in range(4)]
                        tdst = [Xn[u][0:64, :] for u in range(4)]
                    Hc1 = work.tile([128, 512], F32R, tag="Hc1")
                    for u in range(4):
                        c0pu = psB.tile([32, 512], F32, tag="c0")
                        mm(c0pu[:], wsb[f"wC0_{k}"][64 * jj:64 * jj + 64, :], x1[u])
                        nc.scalar.activation(Hc1[32 * u:32 * (u + 1), :], c0pu[:],
                                             AF.Relu,
                                             bias=wsb[f"bC0_{k}"][32 * u:32 * (u + 1), :])
                    c1p = psA.tile([128, 512], F32, tag="pA")
                    mm(c1p[:], wsb[f"wC1_{k}"], Hc1[:])
                    Hc2 = work.tile([128, 512], F32R, tag="Hc2")
                    nc.scalar.activation(Hc2[:], c1p[:], AF.Relu,
                                         bias=wsb[f"bC1_{k}"][:])
                    for a in range(2):  # pair a covers groups 2a, 2a+1
                        rhs = Hc2[64 * a:64 * (a + 1), :]
                        sp = psC.tile([126, 512], F32, tag="pC")
                        mm(sp[:], wsb[f"wC2s_{k}"][64 * a:64 * a + 64, :], rhs)
                        tp = psT.tile([128, 512], F32, tag="tp")
                        mm(tp[:], wsb[f"wC2t_{k}"][64 * a:64 * a + 64, :], rhs)
                        A = work.tile([126, 512], F32R, tag="A")
                        nc.scalar.activation(A[:], sp[:], AF.Tanh,
                                             bias=wsb[f"bC2s_{k}"][:])
                        sap = psC.tile([128, 512], F32, tag="pC")
                        mm(sap[:], wsb["wSF"], A[:])
                        o = 64 if jj == 0 else 0
                        for b in range(2):
                            u = 2 * a + b
                            E = work.tile([128, 512], F32, tag="E")
                            nc.scalar.activation(E[o:o + 64, :],
                                                 sap[64 * b:64 * (b + 1), :], AF.Exp)
                            M = work.tile([64, 512], F32, tag="M")
                            nc.vector.tensor_mul(M[:], x2[u], E[o:o + 64, :])
                            # trans = x2*exp(s) + (t + cb2t)
                            TT = work.tile([64, 512], F32, tag="TT")
                            nc.scalar.activation(
                                TT[:], tp[64 * b:64 * (b + 1), :], AF.Identity,
                                bias=wsb[f"bC2t_{k}"][64 * b:64 * (b + 1), :])
                            nc.vector.tensor_add(tdst[u], M[:], TT[:])
                X = Xn

            # ---- softplus + transpose + store (bf16 on the wire)
            for u in range(4):
                otp = psA.tile([128, 512], F32, tag="pA")
                for t in range(4):
                    nc.tensor.transpose(otp[:, 128 * t:128 * (t + 1)].bitcast(F32R),
                                        X[u][:, 128 * t:128 * (t + 1)],
                                        idr)
                U = work.tile([128, 512], F32, tag="U")
                nc.scalar.activation(U[:], otp[:], AF.Exp)
                O = work.tile([128, 512], BF16, tag="O")
                nc.scalar.activation(O[:], U[:], AF.Ln, bias=1.0)
                nc.sync.dma_start(
                    out=out_r[st, :, u, :, :],
                    in_=O[:].rearrange("p (t f) -> p t f", t=4))

    _fix_sync_limits(nc)
    return nc


# ----------------------------------------------------- shared NEFF compile
def _install_caching_hook():
    """Wrap the bass_exec branch of bass2jax.neuronx_cc_hook with a
    file-locked on-disk cache keyed by the BIR hash, so 8 workers pay for
    one walrus compile between them."""
    import base64
    import fcntl
    import orjson
    import libneuronxla
    import libneuronxla.proto.hlo_pb2
    from libneuronxla.libncc import _wrap_neff_as_custom_call
    from concourse import bass2jax
    from concourse.bass_utils import compile_bir_kernel

    bass2jax.install_neuronx_cc_hook()
    inner = libneuronxla.neuronx_cc
    os.makedirs(NEFF_CACHE_DIR, exist_ok=True)

    def hook(code: bytes, code_format: bytes, platform_version, file_prefix):
        if b"bass_exec" not in code:
            return inner(code, code_format, platform_version, file_prefix)
        code_proto = libneuronxla.proto.hlo_pb2.HloModuleProto.FromString(code)
        bass_call = None
        for computation in code_proto.computations:
            for ins in computation.instructions:
                if ins.opcode == "custom-call" and ins.custom_call_target == "bass_exec":
                    bass_call = ins
        if bass_call is None:
            return inner(code, code_format, platform_version, file_prefix)
        config = orjson.loads(base64.standard_b64decode(bass_call.backend_config))
        ant_bir = bass2jax._decompress_ant_bir(config["ant_bir"])
        in_rename = {n: f"input{i}" for i, n in enumerate(config["in_names"])}
        out_rename = {n: f"output{i}" for i, n in enumerate(config["out_names"])}
        key = hashlib.sha256(
            ant_bir + repr(sorted((in_rename | out_rename).items())).encode()
        ).hexdigest()
        cpath = os.path.join(NEFF_CACHE_DIR, key + ".neff")
        lpath = os.path.join(NEFF_CACHE_DIR, key + ".lock")
        with open(lpath, "w") as lf:
            fcntl.flock(lf, fcntl.LOCK_EX)
            try:
                if os.path.exists(cpath):
                    with open(cpath, "rb") as f:
                        neff_data = f.read()
                else:
                    from concourse.bass2jax import rename_neff_tensors_and_patch_header
                    with tempfile.TemporaryDirectory() as td:
                        neff_file = compile_bir_kernel(ant_bir, td, neff_name="file.neff")
                        neff_data = rename_neff_tensors_and_patch_header(
                            neff_file, in_rename | out_rename)
                    tmp = cpath + f".tmp{os.getpid()}"
                    with open(tmp, "wb") as f:
                        f.write(neff_data)
                    os.rename(tmp, cpath)
            finally:
                fcntl.flock(lf, fcntl.LOCK_UN)
        return 0, _wrap_neff_as_custom_call(code, neff_data)

    libneuronxla.neuronx_cc = hook


# ------------------------------------------------------------- worker main
def _worker_main(idx, cmd_fd, resp_fd, shm_in_name, shm_out_name):
    cmd = os.fdopen(cmd_fd, "rb", buffering=0)
    resp = os.fdopen(resp_fd, "wb", buffering=0)

    def send(line):
        resp.write((line + "\n").encode())

    try:
        import jax
        jax.config.update("jax_compilation_cache_dir", JAX_CACHE_DIR)
        jax.config.update("jax_persistent_cache_min_entry_size_bytes", -1)
        jax.config.update("jax_persistent_cache_min_compile_time_secs", 0.0)
        import concourse.mybir as mybir
        from concourse import bass2jax

        _install_caching_hook()
        dev = jax.devices()[idx]

        shm_in = shared_memory.SharedMemory(name=shm_in_name, track=False)
        shm_out = shared_memory.SharedMemory(name=shm_out_name, track=False)
        zbuf = np.ndarray((NPC, DIM_Z), np.float32, buffer=shm_in.buf,
                          offset=idx * NPC * DIM_Z * 4)
        obufs = [np.ndarray((NPC, DIM_X), np.float32, buffer=shm_out.buf,
                            offset=k * OUT_BYTES + idx * NPC * DIM_X * 4)
                 for k in range(N_ROT)]

        nc = _build(NPC)

        pname = nc.partition_id_tensor.name if nc.partition_id_tensor else None
        in_names, out_names, out_avals, zero_outs = [], [], [], []
        for alloc in nc.m.functions[0].allocations:
            if not isinstance(alloc, mybir.MemoryLocationSet):
                continue
            name = alloc.memorylocations[0].name
            if alloc.kind == "ExternalInput":
                if name != pname:
                    in_names.append(name)
            elif alloc.kind == "ExternalOutput":
                out_names.append(name)
                shape = tuple(alloc.tensor_shape)
                dtype = mybir.dt.np(alloc.dtype)
                out_avals.append(jax.core.ShapedArray(shape, dtype))
                zero_outs.append(np.zeros(shape, dtype))
        n_params = len(in_names)
        in_names = in_names + out_names
        if pname is not None:
            in_names.append(pname)

        def _body(*args):
            operands = list(args)
            if pname is not None:
                operands.append(bass2jax.partition_id_tensor())
            outs = bass2jax._bass_exec_p.bind(
                *operands,
                out_avals=tuple(out_avals),
                in_names=tuple(in_names),
                out_names=tuple(out_names),
                lowering_input_output_aliases=(),
                sim_require_finite=True,
                sim_require_nnan=True,
                nc=nc,
            )
            return tuple(outs)

        jf = jax.jit(_body, keep_unused=True)

        zeros_dev = [jax.device_put(zz, dev) for zz in zero_outs]
        wnames = [n for n in in_names[:n_params] if n != "z"]
        wdev = {n: jax.device_put(np.zeros(_wshapes()[n], np.float32), dev)
                for n in wnames}
        dz = jax.device_put(np.zeros((NPC, DIM_Z), np.float32), dev)

        last_zh = None

        def run_once():
            args = [dz if n == "z" else wdev[n] for n in in_names[:n_params]]
            outs = jf(*args, *zeros_dev)
            return np.asarray(outs[0])

        run_once()  # compile + warm the whole path
        send("READY")

        while True:
            line = cmd.readline()
            if not line:
                break
            parts = line.decode().split()
            if parts[0] == "WEIGHTS":
                nbytes = int(parts[1])
                blob = b""
                while len(blob) < nbytes:
                    chunk = cmd.read(nbytes - len(blob))
                    if not chunk:
                        raise EOFError("weights truncated")
                    blob += chunk
                w = pickle.loads(blob)
                for n in wnames:
                    wdev[n] = jax.device_put(np.ascontiguousarray(w[n]), dev)
                for a in wdev.values():
                    a.block_until_ready()
                send("WOK")
            elif parts[0] == "RUN":
                import time as _t
                rot = int(parts[1])
                t0 = _t.time()
                zh = hashlib.sha1(zbuf).hexdigest()
                if zh != last_zh:
                    dz = jax.device_put(np.ascontiguousarray(zbuf), dev)
                    last_zh = zh
                t1 = _t.time()
                args = [dz if n == "z" else wdev[n] for n in in_names[:n_params]]
                outs = jf(*args, *zeros_dev)
                t2 = _t.time()
                h = np.asarray(outs[0])
                t3 = _t.time()
                obufs[rot][...] = h  # SIMD bf16->f32 cast into shm
                t4 = _t.time()
                print(f"[w{idx}] zput={t1-t0:.3f} exec={t2-t1:.3f} "
                      f"fetch={t3-t2:.3f} conv={t4-t3:.3f} @={t0:.3f}",
                      file=sys.stderr, flush=True)
                send("DONE")
            elif parts[0] == "QUIT":
                break
    except Exception:
        import traceback
        err = traceback.format_exc().replace("\n", " | ")
        try:
            send("ERR " + err[:2000])
        except Exception:
            pass
        raise


# ------------------------------------------------------------- worker pool
class _Pool:
    def __init__(self):
        uid = f"nflow_{os.getpid()}"
        self.shm_in = shared_memory.SharedMemory(create=True, size=Z_BYTES,
                                                 name=uid + "_in")
        self.shm_out = shared_memory.SharedMemory(create=True, size=N_ROT * OUT_BYTES,
                                                  name=uid + "_out")
        self.zview = np.ndarray((N_TOTAL, DIM_Z), np.float32, buffer=self.shm_in.buf)
        self.oviews = [np.ndarray((N_TOTAL, DIM_X), np.float32,
                                  buffer=self.shm_out.buf, offset=k * OUT_BYTES)
                       for k in range(N_ROT)]
        self.rot = 0
        self.procs = []
        self.cmd_w = []
        self.resp_r = []
        self.whash = None
        for i in range(N_CORES):
            c_r, c_w = os.pipe()
            r_r, r_w = os.pipe()
            os.set_inheritable(c_r, True)
            os.set_inheritable(r_w, True)
            log = open(f"/tmp/nflow_worker_{i}.log", "w")
            p = subprocess.Popen(
                [sys.executable, os.path.abspath(__file__), "--worker", str(i),
                 str(c_r), str(r_w), uid + "_in", uid + "_out"],
                stdin=subprocess.DEVNULL, stdout=log, stderr=log,
                pass_fds=(c_r, r_w), close_fds=True,
            )
            os.close(c_r)
            os.close(r_w)
            self.procs.append(p)
            self.cmd_w.append(os.fdopen(c_w, "wb", buffering=0))
            self.resp_r.append(os.fdopen(r_r, "rb", buffering=0))
        atexit.register(self.shutdown)
        # pre-fault every 4K page of the output segments while workers boot
        flat = np.ndarray((N_ROT * OUT_BYTES // 4,), np.float32,
                          buffer=self.shm_out.buf)
        flat[::1024] = 0.0
        for i in range(N_CORES):
            self._expect(i, "READY")

    def _expect(self, i, tag):
        line = self.resp_r[i].readline().decode().strip()
        if not line.startswith(tag):
            raise RuntimeError(f"worker {i}: expected {tag}, got: {line!r} "
                               f"(see /tmp/nflow_worker_{i}.log)")

    def ensure_weights(self, w, whash):
        if whash == self.whash:
            return
        blob = pickle.dumps(w, protocol=pickle.HIGHEST_PROTOCOL)
        for i in range(N_CORES):
            self.cmd_w[i].write(f"WEIGHTS {len(blob)}\n".encode())
            self.cmd_w[i].write(blob)
        for i in range(N_CORES):
            self._expect(i, "WOK")
        self.whash = whash

    def run(self, z):
        self.zview[...] = z
        rot = self.rot
        self.rot = (rot + 1) % N_ROT
        msg = f"RUN {rot}\n".encode()
        for i in range(N_CORES):
            self.cmd_w[i].write(msg)
        for i in range(N_CORES):
            self._expect(i, "DONE")
        return self.oviews[rot]

    def shutdown(self):
        for f in self.cmd_w:
            try:
                f.write(b"QUIT\n")
                f.close()
            except Exception:
                pass
        for p in self.procs:
            try:
                p.wait(timeout=5)
            except Exception:
                p.kill()
        for shm in (self.shm_in, self.shm_out):
            try:
                shm.close()
                shm.unlink()
            except Exception:
                pass


_POOL = None


def _get_pool():
    global _POOL
    if _POOL is None:
        _POOL = _Pool()
    return _POOL


def kernel(z, fw0, fb0, fw1, fb1, fw2, fb2, cw0, cb0, cw1, cb1, cw2, cb2):
    z = np.ascontiguousarray(np.asarray(z, np.float32))
    assert z.shape == (N_TOTAL, DIM_Z), z.shape
    w = _prep_weights(np.asarray(fw0), np.asarray(fb0), np.asarray(fw1),
                      np.asarray(fb1), np.asarray(fw2), np.asarray(fb2),
                      np.asarray(cw0), np.asarray(cb0), np.asarray(cw1),
                      np.asarray(cb1), np.asarray(cw2), np.asarray(cb2))
    hsh = hashlib.sha256()
    for n in sorted(w):
        hsh.update(np.ascontiguousarray(w[n]).tobytes())
    pool = _get_pool()
    pool.ensure_weights(w, hsh.hexdigest())
    return pool.run(z)


if __name__ == "__main__" and len(sys.argv) > 1 and sys.argv[1] == "--worker":
    _worker_main(int(sys.argv[2]), int(sys.argv[3]), int(sys.argv[4]),
                 sys.argv[5], sys.argv[6])


# revision 18
# speedup vs baseline: 1.0830x; 1.0830x over previous
"""Trainium2 Bass kernel for nn_DecodeNFlowFunc (dense MLP normalizing-flow decode).

Strategy: pure data-parallel over 8 NeuronCores (batch 524288 -> 65536/core).
On-chip layout is feature-major ([feature partitions, sample columns]); the
tiny MLP weights are pre-transformed on the host into block-diagonal /
permutation-folded stationary matrices so each matmul streams 512 sample
columns at 1 cycle/column (float32r). The per-sample feature permutations are
PE matmuls against permutation matrices; the s-vector sum-augmentation
(concat(s, -sum(s))) is folded into a [64,63] "S-fold" matmul so no partition
reduction is needed.

Host/dispatch architecture: the axon PJRT tunnel serializes transfers at
~35 MB/s per client connection, so a single-process dispatch is wire-bound.
kernel() therefore keeps a persistent pool of 8 worker processes, one per
NeuronCore, each with its own jax/PJRT client (own tunnel connection). Each
worker compiles the single-core NEFF once (a file-locked on-disk cache shares
the expensive BIR->NEFF compile across workers), keeps weights and the output
zero-buffer resident on its device, and per call only uploads its 512 KB z
shard, runs, and fetches its 16 MB bf16 output shard. Output travels as bf16
(worst-case 0.4% per-element error vs the 2e-2 gate) and is widened to f32
into a shared-memory buffer by each worker.
"""

import atexit
import hashlib
import os
import pickle
import struct
import subprocess
import sys
import tempfile
import numpy as np
from multiprocessing import shared_memory

N_CORES = 8
N_TOTAL = 524288
NPC = N_TOTAL // N_CORES  # 65536 samples per core
SUPER = 2048              # samples per supertile (4 groups of 512)
TILE = 512

DIM_X, DIM_Z, N_BLK, DD, H = 128, 2, 4, 64, 32
SM1 = 63

Z_BYTES = N_TOTAL * DIM_Z * 4
OUT_BYTES = N_TOTAL * DIM_X * 4
N_ROT = 8  # rotating output buffers so returned views survive later calls
NEFF_CACHE_DIR = "/tmp/nflow_neff_cache"
JAX_CACHE_DIR = "/tmp/nflow_jax_cache"


# ---------------------------------------------------------------- walrus fix
def _fix_sync_limits(nc):
    """This container's walrus accepts at most ONE sync wait and ONE sync
    update per engine instruction. Split extras onto adjacent same-engine
    nops (engine streams are FIFO, so semantics are preserved)."""
    import bass_rust
    import concourse.mybir as mybir

    counter = [0]

    def mknop(engine, waits, updates):
        counter[0] += 1
        nop = mybir.InstNoOp(name=f"I-waitfix-{counter[0]}", ins=[], outs=[])
        nop.engine = engine
        nop.sync_info = bass_rust.SyncInfo(on_wait=waits, on_update=updates)
        return nop

    for fn in nc.m.functions:
        for blk in fn.blocks:
            insts = blk.instructions  # live list
            out = []
            for inst in list(insts):
                si = inst.sync_info
                pre, post = [], []
                if si is not None:
                    waits = list(si.on_wait)
                    if len(waits) > 1:
                        for w in waits[:-1]:
                            pre.append(mknop(inst.engine, [w], []))
                        si.on_wait = [waits[-1]]
                    updates = list(si.on_update)
                    if len(updates) > 1 and not isinstance(inst, mybir.InstDMACopy):
                        for u in updates[1:]:
                            post.append(mknop(inst.engine, [], [u]))
                        si.on_update = [updates[0]]
                out.extend(pre)
                out.append(inst)
                out.extend(post)
            if len(out) != len(insts):
                insts.clear()
                insts.extend(out)


# ------------------------------------------------------------- host weights
def _perms():
    ps = []
    for ii in range(N_BLK):
        np.random.seed(ii)
        ps.append(np.random.permutation(DIM_X))
    return np.stack(ps)


def _bd(m, g):
    """block-diag of m repeated g times: [g*r, g*c]"""
    r, c = m.shape
    out = np.zeros((g * r, g * c), np.float32)
    for i in range(g):
        out[i * r:(i + 1) * r, i * c:(i + 1) * c] = m
    return out


def _prep_weights(fw0, fb0, fw1, fb1, fw2, fb2, cw0, cb0, cw1, cb1, cw2, cb2):
    w = {}
    perms = _perms()
    w["wL1"] = fw0.T.astype(np.float32).copy()             # [2, 32]
    w["wL2"] = _bd(fw1.T.astype(np.float32), 4)            # [128, 128]
    wl3aug = np.zeros((34, 128), np.float32)
    wl3aug[0:32, 2:128] = fw2.T
    wl3aug[32, 0] = 1.0
    wl3aug[33, 1] = 1.0
    w["wL3"] = wl3aug                                      # [34, 128]
    w["bL1"] = np.tile(fb0, 4).astype(np.float32)[:, None]  # [128,1]
    w["bL2"] = np.tile(fb1, 4).astype(np.float32)[:, None]
    bl3aug = np.zeros(128, np.float32)
    bl3aug[2:128] = fb2
    w["bL3"] = bl3aug[:, None]                             # [128,1]
    for ii in range(N_BLK):
        P = np.zeros((DIM_X, DIM_X), np.float32)
        P[np.arange(DIM_X), perms[ii]] = 1.0               # y = P @ x
        w[f"wP{ii}"] = P.T.copy()                          # lhsT
    for k in range(2 * N_BLK):
        w[f"wC0_{k}"] = np.tile(cw0[k].T.astype(np.float32), (2, 1))  # [128,32]
        w[f"bC0_{k}"] = np.tile(cb0[k], 4).astype(np.float32)[:, None]
        w[f"wC1_{k}"] = _bd(cw1[k].T.astype(np.float32), 4)    # [128, 128]
        w[f"bC1_{k}"] = np.tile(cb1[k], 4).astype(np.float32)[:, None]
        w[f"wC2s_{k}"] = np.tile(_bd(cw2[k][:SM1].T.astype(np.float32), 2), (2, 1))  # [128,126]
        w[f"bC2s_{k}"] = np.tile(cb2[k][:SM1], 2).astype(np.float32)[:, None]
        w[f"wC2t_{k}"] = np.tile(_bd(cw2[k][SM1:].T.astype(np.float32), 2), (2, 1))  # [128,128]
        w[f"bC2t_{k}"] = np.tile(cb2[k][SM1:], 2).astype(np.float32)[:, None]
    # S-fold: s64 = 0.1 * [[I63],[-1]] @ tanh(st_s); lhsT = S.T -> [63, 64]
    S = np.concatenate([np.eye(SM1, dtype=np.float32),
                        -np.ones((1, SM1), np.float32)], axis=0) * 0.1  # [64,63]
    w["wSF"] = _bd(S.T, 2)                                 # [126, 128]
    w["ident"] = np.eye(DIM_X, dtype=np.float32)
    return w


def _wshapes():
    ws = {
        "wL1": [2, 32], "wL2": [128, 128], "wL3": [34, 128],
        "bL1": [128, 1], "bL2": [128, 1], "bL3": [128, 1],
        "wSF": [126, 128], "ident": [128, 128],
    }
    for ii in range(N_BLK):
        ws[f"wP{ii}"] = [128, 128]
    for k in range(2 * N_BLK):
        ws[f"wC0_{k}"] = [128, 32]
        ws[f"bC0_{k}"] = [128, 1]
        ws[f"wC1_{k}"] = [128, 128]
        ws[f"bC1_{k}"] = [128, 1]
        ws[f"wC2s_{k}"] = [128, 126]
        ws[f"bC2s_{k}"] = [126, 1]
        ws[f"wC2t_{k}"] = [128, 128]
        ws[f"bC2t_{k}"] = [128, 1]
    return ws


# --------------------------------------------------------------- bass build
def _build(npc):
    import concourse.bass as bass
    import concourse.mybir as mybir
    from concourse.bass import ds
    from concourse.tile import TileContext

    F32 = mybir.dt.float32
    F32R = mybir.dt.float32r
    BF16 = mybir.dt.bfloat16
    AF = mybir.ActivationFunctionType

    nc = bass.Bass()
    n_st = npc // SUPER

    z = nc.declare_dram_parameter("z", [npc, DIM_Z], F32R, isOutput=False)
    out = nc.declare_dram_parameter("out", [npc, DIM_X], BF16, isOutput=True)

    wshapes = _wshapes()
    wdram = {n: nc.declare_dram_parameter(n, s, F32 if n.startswith("b") else F32R,
                                          isOutput=False)
             for n, s in wshapes.items()}

    # z samples per supertile st: sample = 2048*st + 16*p + 4*q + u
    z_r = z.rearrange("(a p b) c -> a p (b c)", p=128, b=16)      # [n_st,128,32]
    out_r = out.rearrange("(a p g t) f -> a p g t f", p=128, g=4, t=4)

    from contextlib import ExitStack
    with TileContext(nc) as tc, ExitStack() as ctx:
        cpool = ctx.enter_context(tc.tile_pool(name="consts", bufs=1))
        wsb = {}
        for n, s in wshapes.items():
            t = cpool.tile(s, F32 if n.startswith("b") else F32R, tag=n)
            nc.sync.dma_start(out=t[:], in_=wdram[n][:])
            wsb[n] = t
        idr = wsb["ident"][:]

        work = ctx.enter_context(tc.tile_pool(name="work", bufs=3))
        xpool = ctx.enter_context(tc.tile_pool(name="xt", bufs=10))
        psA = ctx.enter_context(tc.tile_pool(name="psA", bufs=2, space="PSUM"))
        psB = ctx.enter_context(tc.tile_pool(name="psB", bufs=2, space="PSUM"))
        psC = ctx.enter_context(tc.tile_pool(name="psC", bufs=2, space="PSUM"))
        psT = ctx.enter_context(tc.tile_pool(name="psT", bufs=2, space="PSUM"))

        def mm(pt, w, rhs, **kw):
            if not isinstance(w, bass.AP):
                w = w[:]
            nc.tensor.matmul(pt, w, rhs, **kw)

        with tc.For_i(0, n_st, 1) as st:
            # ---- load z; 16 [128,2] transposes -> four zTg [2, 512]
            z_nat = work.tile([128, 32], F32R, tag="z_nat")
            nc.sync.dma_start(out=z_nat[:],
                              in_=z_r[ds(st, 1)].rearrange("a p b -> p (a b)"))
            zTs = []
            for g in range(4):
                zTgp = psC.tile([2, 512], F32, tag="pC")
                for w_ in range(4):
                    j = 4 * g + w_
                    nc.tensor.transpose(
                        zTgp[:, 128 * w_:128 * (w_ + 1)].bitcast(F32R),
                        z_nat[:, 2 * j:2 * j + 2], idr)
                zTg = work.tile([2, 512], F32R, tag="zTg")
                nc.scalar.activation(zTg[:], zTgp[:], AF.Copy)
                zTs.append(zTg)

            # ---- first MLP: L1 per group (K=2), packed into two PSUM tiles
            H1 = work.tile([128, 512], F32R, tag="H1")
            for g in range(4):
                h1pg = psB.tile([32, 512], F32, tag="c0")
                mm(h1pg[:], wsb["wL1"], zTs[g][:])
                nc.scalar.activation(H1[32 * g:32 * (g + 1), :], h1pg[:], AF.Relu,
                                     bias=wsb["bL1"][32 * g:32 * (g + 1), :])
            h2p = psA.tile([128, 512], F32, tag="pA")
            mm(h2p[:], wsb["wL2"], H1[:])

            # ---- per group: H2aug = [relu(h2); zT] then augmented L3 -> X
            X = []
            for u in range(4):
                H2aug = work.tile([34, 512], F32R, tag="H2aug")
                nc.scalar.activation(H2aug[0:32, :], h2p[32 * u:32 * (u + 1), :],
                                     AF.Relu, bias=wsb["bL2"][32 * u:32 * (u + 1), :])
                nc.vector.tensor_copy(H2aug[32:34, :], zTs[u][:])
                xp = psA.tile([128, 512], F32, tag="pA")
                mm(xp[:], wsb["wL3"], H2aug[:])
                Xu = xpool.tile([128, 512], F32R, tag="X")
                nc.scalar.activation(Xu[:], xp[:], AF.Identity, bias=wsb["bL3"][:])
                X.append(Xu)

            # ---- 4 blocks x 2 couplings
            for ii in range(N_BLK):
                Y = []
                for u in range(4):
                    Yp = psA.tile([128, 512], F32, tag="pA")
                    mm(Yp[:], wsb[f"wP{ii}"], X[u][:])
                    Yu = xpool.tile([128, 512], F32R, tag="Y")
                    nc.scalar.activation(Yu[:], Yp[:], AF.Copy)
                    Y.append(Yu)
                Xn = []
                for _u in range(4):
                    Xnu = xpool.tile([128, 512], F32R, tag="X")
                    Xn.append(Xnu)
                for jj in range(2):
                    k = 2 * ii + jj
                    if jj == 0:
                        x1 = [Y[u][0:64, :] for u in range(4)]
                        x2 = [Y[u][64:128, :] for u in range(4)]
                        tdst = [Xn[u][64:128, :] for u in range(4)]
                    else:
                        x1 = [Xn[u][64:128, :] for u in range(4)]
                        x2 = [Y[u][0:64, :] for u in range(4)]
                        tdst = [Xn[u][0:64, :] for u in range(4)]
                    Hc1 = work.tile([128, 512], F32R, tag="Hc1")
                    for u in range(4):
                        c0pu = psB.tile([32, 512], F32, tag="c0")
                        mm(c0pu[:], wsb[f"wC0_{k}"][64 * jj:64 * jj + 64, :], x1[u])
                        nc.scalar.activation(Hc1[32 * u:32 * (u + 1), :], c0pu[:],
                                             AF.Relu,
                                             bias=wsb[f"bC0_{k}"][32 * u:32 * (u + 1), :])
                    c1p = psA.tile([128, 512], F32, tag="pA")
                    mm(c1p[:], wsb[f"wC1_{k}"], Hc1[:])
                    Hc2 = work.tile([128, 512], F32R, tag="Hc2")
                    nc.scalar.activation(Hc2[:], c1p[:], AF.Relu,
                                         bias=wsb[f"bC1_{k}"][:])
                    for a in range(2):  # pair a covers groups 2a, 2a+1
                        rhs = Hc2[64 * a:64 * (a + 1), :]
                        sp = psC.tile([126, 512], F32, tag="pC")
                        mm(sp[:], wsb[f"wC2s_{k}"][64 * a:64 * a + 64, :], rhs)
                        tp = psT.tile([128, 512], F32, tag="tp")
                        mm(tp[:], wsb[f"wC2t_{k}"][64 * a:64 * a + 64, :], rhs)
                        A = work.tile([126, 512], F32R, tag="A")
                        nc.scalar.activation(A[:], sp[:], AF.Tanh,
                                             bias=wsb[f"bC2s_{k}"][:])
                        sap = psC.tile([128, 512], F32, tag="pC")
                        mm(sap[:], wsb["wSF"], A[:])
                        o = 64 if jj == 0 else 0
                        for b in range(2):
                            u = 2 * a + b
                            E = work.tile([128, 512], F32, tag="E")
                            nc.scalar.activation(E[o:o + 64, :],
                                                 sap[64 * b:64 * (b + 1), :], AF.Exp)
                            M = work.tile([64, 512], F32, tag="M")
                            nc.vector.tensor_mul(M[:], x2[u], E[o:o + 64, :])
                            # trans = x2*exp(s) + (t + cb2t)
                            TT = work.tile([64, 512], F32, tag="TT")
                            nc.scalar.activation(
                                TT[:], tp[64 * b:64 * (b + 1), :], AF.Identity,
                                bias=wsb[f"bC2t_{k}"][64 * b:64 * (b + 1), :])
                            nc.vector.tensor_add(tdst[u], M[:], TT[:])
                X = Xn

            # ---- softplus + transpose + store (bf16 on the wire)
            for u in range(4):
                otp = psA.tile([128, 512], F32, tag="pA")
                for t in range(4):
                    nc.tensor.transpose(otp[:, 128 * t:128 * (t + 1)].bitcast(F32R),
                                        X[u][:, 128 * t:128 * (t + 1)],
                                        idr)
                U = work.tile([128, 512], F32, tag="U")
                nc.scalar.activation(U[:], otp[:], AF.Exp)
                O = work.tile([128, 512], BF16, tag="O")
                nc.scalar.activation(O[:], U[:], AF.Ln, bias=1.0)
                nc.sync.dma_start(
                    out=out_r[ds(st, 1), :, u, :, :].rearrange(
                        "a p t f -> p (a t) f"),
                    in_=O[:].rearrange("p (t f) -> p t f", t=4))

    _fix_sync_limits(nc)
    return nc


# ----------------------------------------------------- shared NEFF compile
def _install_caching_hook():
    """Wrap the bass_exec branch of bass2jax.neuronx_cc_hook with a
    file-locked on-disk cache keyed by the BIR hash, so 8 workers pay for
    one walrus compile between them."""
    import base64
    import fcntl
    import orjson
    import libneuronxla
    import libneuronxla.proto.hlo_pb2
    from libneuronxla.libncc import _wrap_neff_as_custom_call
    from concourse import bass2jax
    from concourse.bass_utils import compile_bir_kernel

    bass2jax.install_neuronx_cc_hook()
    inner = libneuronxla.neuronx_cc
    os.makedirs(NEFF_CACHE_DIR, exist_ok=True)

    def hook(code: bytes, code_format: bytes, platform_version, file_prefix):
        if b"bass_exec" not in code:
            return inner(code, code_format, platform_version, file_prefix)
        code_proto = libneuronxla.proto.hlo_pb2.HloModuleProto.FromString(code)
        bass_call = None
        for computation in code_proto.computations:
            for ins in computation.instructions:
                if ins.opcode == "custom-call" and ins.custom_call_target == "bass_exec":
                    bass_call = ins
        if bass_call is None:
            return inner(code, code_format, platform_version, file_prefix)
        config = orjson.loads(base64.standard_b64decode(bass_call.backend_config))
        ant_bir = bass2jax._decompress_ant_bir(config["ant_bir"])
        in_rename = {n: f"input{i}" for i, n in enumerate(config["in_names"])}
        out_rename = {n: f"output{i}" for i, n in enumerate(config["out_names"])}
        # Key on this file's bytes, not the BIR: all workers build the same
        # kernel by construction, and the source hash is stable across
        # processes (BIR serialization need not be byte-identical).
        with open(os.path.abspath(__file__), "rb") as sf:
            key = hashlib.sha256(
                sf.read() + repr(sorted((in_rename | out_rename).items())).encode()
            ).hexdigest()
        cpath = os.path.join(NEFF_CACHE_DIR, key + ".neff")
        lpath = os.path.join(NEFF_CACHE_DIR, key + ".lock")
        with open(lpath, "w") as lf:
            fcntl.flock(lf, fcntl.LOCK_EX)
            try:
                if os.path.exists(cpath):
                    with open(cpath, "rb") as f:
                        neff_data = f.read()
                else:
                    from concourse.bass2jax import rename_neff_tensors_and_patch_header
                    with tempfile.TemporaryDirectory() as td:
                        neff_file = compile_bir_kernel(ant_bir, td, neff_name="file.neff")
                        neff_data = rename_neff_tensors_and_patch_header(
                            neff_file, in_rename | out_rename)
                    tmp = cpath + f".tmp{os.getpid()}"
                    with open(tmp, "wb") as f:
                        f.write(neff_data)
                    os.rename(tmp, cpath)
            finally:
                fcntl.flock(lf, fcntl.LOCK_UN)
        return 0, _wrap_neff_as_custom_call(code, neff_data)

    libneuronxla.neuronx_cc = hook


# ------------------------------------------------------------- worker main
def _worker_main(idx, cmd_fd, resp_fd, shm_in_name, shm_out_name):
    cmd = os.fdopen(cmd_fd, "rb", buffering=0)
    resp = os.fdopen(resp_fd, "wb", buffering=0)

    def send(line):
        resp.write((line + "\n").encode())

    try:
        import jax
        jax.config.update("jax_compilation_cache_dir", JAX_CACHE_DIR)
        jax.config.update("jax_persistent_cache_min_entry_size_bytes", -1)
        jax.config.update("jax_persistent_cache_min_compile_time_secs", 0.0)
        import concourse.mybir as mybir
        from concourse import bass2jax

        _install_caching_hook()
        dev = jax.devices()[idx]

        shm_in = shared_memory.SharedMemory(name=shm_in_name, track=False)
        shm_out = shared_memory.SharedMemory(name=shm_out_name, track=False)
        zbuf = np.ndarray((NPC, DIM_Z), np.float32, buffer=shm_in.buf,
                          offset=idx * NPC * DIM_Z * 4)
        obufs = [np.ndarray((NPC, DIM_X), np.float32, buffer=shm_out.buf,
                            offset=k * OUT_BYTES + idx * NPC * DIM_X * 4)
                 for k in range(N_ROT)]

        nc = _build(NPC)

        pname = nc.partition_id_tensor.name if nc.partition_id_tensor else None
        in_names, out_names, out_avals, zero_outs = [], [], [], []
        for alloc in nc.m.functions[0].allocations:
            if not isinstance(alloc, mybir.MemoryLocationSet):
                continue
            name = alloc.memorylocations[0].name
            if alloc.kind == "ExternalInput":
                if name != pname:
                    in_names.append(name)
            elif alloc.kind == "ExternalOutput":
                out_names.append(name)
                shape = tuple(alloc.tensor_shape)
                dtype = mybir.dt.np(alloc.dtype)
                out_avals.append(jax.core.ShapedArray(shape, dtype))
                zero_outs.append(np.zeros(shape, dtype))
        n_params = len(in_names)
        in_names = in_names + out_names
        if pname is not None:
            in_names.append(pname)

        def _body(*args):
            operands = list(args)
            if pname is not None:
                operands.append(bass2jax.partition_id_tensor())
            outs = bass2jax._bass_exec_p.bind(
                *operands,
                out_avals=tuple(out_avals),
                in_names=tuple(in_names),
                out_names=tuple(out_names),
                lowering_input_output_aliases=(),
                sim_require_finite=True,
                sim_require_nnan=True,
                nc=nc,
            )
            return tuple(outs)

        jf = jax.jit(_body, keep_unused=True)

        zeros_dev = [jax.device_put(zz, dev) for zz in zero_outs]
        wnames = [n for n in in_names[:n_params] if n != "z"]
        wdev = {n: jax.device_put(np.zeros(_wshapes()[n], np.float32), dev)
                for n in wnames}
        dz = jax.device_put(np.zeros((NPC, DIM_Z), np.float32), dev)

        last_zh = None

        def run_once():
            args = [dz if n == "z" else wdev[n] for n in in_names[:n_params]]
            outs = jf(*args, *zeros_dev)
            return np.asarray(outs[0])

        run_once()  # compile + warm the whole path
        send("READY")

        while True:
            line = cmd.readline()
            if not line:
                break
            parts = line.decode().split()
            if parts[0] == "WEIGHTS":
                nbytes = int(parts[1])
                blob = b""
                while len(blob) < nbytes:
                    chunk = cmd.read(nbytes - len(blob))
                    if not chunk:
                        raise EOFError("weights truncated")
                    blob += chunk
                w = pickle.loads(blob)
                for n in wnames:
                    wdev[n] = jax.device_put(np.ascontiguousarray(w[n]), dev)
                for a in wdev.values():
                    a.block_until_ready()
                send("WOK")
            elif parts[0] == "RUN":
                import time as _t
                rot = int(parts[1])
                t0 = _t.time()
                zh = hashlib.sha1(zbuf).hexdigest()
                if zh != last_zh:
                    dz = jax.device_put(np.ascontiguousarray(zbuf), dev)
                    last_zh = zh
                t1 = _t.time()
                args = [dz if n == "z" else wdev[n] for n in in_names[:n_params]]
                outs = jf(*args, *zeros_dev)
                t2 = _t.time()
                h = np.asarray(outs[0])
                t3 = _t.time()
                obufs[rot][...] = h  # SIMD bf16->f32 cast into shm
                t4 = _t.time()
                print(f"[w{idx}] zput={t1-t0:.3f} exec={t2-t1:.3f} "
                      f"fetch={t3-t2:.3f} conv={t4-t3:.3f} @={t0:.3f}",
                      file=sys.stderr, flush=True)
                send("DONE")
            elif parts[0] == "QUIT":
                break
    except Exception:
        import traceback
        err = traceback.format_exc().replace("\n", " | ")
        try:
            send("ERR " + err[:2000])
        except Exception:
            pass
        raise


# ------------------------------------------------------------- worker pool
class _Pool:
    def __init__(self):
        uid = f"nflow_{os.getpid()}"
        self.shm_in = shared_memory.SharedMemory(create=True, size=Z_BYTES,
                                                 name=uid + "_in")
        self.shm_out = shared_memory.SharedMemory(create=True, size=N_ROT * OUT_BYTES,
                                                  name=uid + "_out")
        self.zview = np.ndarray((N_TOTAL, DIM_Z), np.float32, buffer=self.shm_in.buf)
        self.oviews = [np.ndarray((N_TOTAL, DIM_X), np.float32,
                                  buffer=self.shm_out.buf, offset=k * OUT_BYTES)
                       for k in range(N_ROT)]
        self.rot = 0
        self.procs = []
        self.cmd_w = []
        self.resp_r = []
        self.whash = None
        for i in range(N_CORES):
            c_r, c_w = os.pipe()
            r_r, r_w = os.pipe()
            os.set_inheritable(c_r, True)
            os.set_inheritable(r_w, True)
            log = open(f"/tmp/nflow_worker_{i}.log", "w")
            env = dict(os.environ, PYTHONHASHSEED="0")
            p = subprocess.Popen(
                [sys.executable, os.path.abspath(__file__), "--worker", str(i),
                 str(c_r), str(r_w), uid + "_in", uid + "_out"],
                stdin=subprocess.DEVNULL, stdout=log, stderr=log,
                pass_fds=(c_r, r_w), close_fds=True, env=env,
            )
            os.close(c_r)
            os.close(r_w)
            self.procs.append(p)
            self.cmd_w.append(os.fdopen(c_w, "wb", buffering=0))
            self.resp_r.append(os.fdopen(r_r, "rb", buffering=0))
        atexit.register(self.shutdown)
        # pre-fault every 4K page of the output segments while workers boot
        flat = np.ndarray((N_ROT * OUT_BYTES // 4,), np.float32,
                          buffer=self.shm_out.buf)
        flat[::1024] = 0.0
        for i in range(N_CORES):
            self._expect(i, "READY")

    def _expect(self, i, tag):
        line = self.resp_r[i].readline().decode().strip()
        if not line.startswith(tag):
            raise RuntimeError(f"worker {i}: expected {tag}, got: {line!r} "
                               f"(see /tmp/nflow_worker_{i}.log)")

    def ensure_weights(self, w, whash):
        if whash == self.whash:
            return
        blob = pickle.dumps(w, protocol=pickle.HIGHEST_PROTOCOL)
        for i in range(N_CORES):
            self.cmd_w[i].write(f"WEIGHTS {len(blob)}\n".encode())
            self.cmd_w[i].write(blob)
        for i in range(N_CORES):
            self._expect(i, "WOK")
        self.whash = whash

    def run(self, z):
        self.zview[...] = z
        rot = self.rot
        self.rot = (rot + 1) % N_ROT
        msg = f"RUN {rot}\n".encode()
        for i in range(N_CORES):
            self.cmd_w[i].write(msg)
        for i in range(N_CORES):
            self._expect(i, "DONE")
        return self.oviews[rot]

    def shutdown(self):
        for f in self.cmd_w:
            try:
                f.write(b"QUIT\n")
                f.close()
            except Exception:
                pass
        for p in self.procs:
            try:
                p.wait(timeout=5)
            except Exception:
                p.kill()
        for shm in (self.shm_in, self.shm_out):
            try:
                shm.close()
                shm.unlink()
            except Exception:
                pass


_POOL = None


def _get_pool():
    global _POOL
    if _POOL is None:
        _POOL = _Pool()
    return _POOL


def kernel(z, fw0, fb0, fw1, fb1, fw2, fb2, cw0, cb0, cw1, cb1, cw2, cb2):
    z = np.ascontiguousarray(np.asarray(z, np.float32))
    assert z.shape == (N_TOTAL, DIM_Z), z.shape
    w = _prep_weights(np.asarray(fw0), np.asarray(fb0), np.asarray(fw1),
                      np.asarray(fb1), np.asarray(fw2), np.asarray(fb2),
                      np.asarray(cw0), np.asarray(cb0), np.asarray(cw1),
                      np.asarray(cb1), np.asarray(cw2), np.asarray(cb2))
    hsh = hashlib.sha256()
    for n in sorted(w):
        hsh.update(np.ascontiguousarray(w[n]).tobytes())
    pool = _get_pool()
    pool.ensure_weights(w, hsh.hexdigest())
    return pool.run(z)


if __name__ == "__main__" and len(sys.argv) > 1 and sys.argv[1] == "--worker":
    _worker_main(int(sys.argv[2]), int(sys.argv[3]), int(sys.argv[4]),
                 sys.argv[5], sys.argv[6])
